# revision 1
# baseline (speedup 1.0000x reference)
"""Trainium2 Bass kernel for nn_Damping: per-sample Cholesky-factor damping.

Math (per sample b):
  h  = tanh MLPs of x0 -> diag xd [64], offdiag z [2016] (strict lower tri of L)
  y  = L^T x0 ; D = L y

Implementation (per core, feature-major layout [feature partitions, batch free]):
  - all matmuls in fp32r (full PE rate at free>=256, ~1e-4 rel err)
  - L matvecs without materializing L, using static 0/1 scatter/gather
    matrices on the tensor engine:
      x0g = R @ x0          (x0g[p] = x0[row(p)])
      y   = C^T (z*x0g) + Q1 x0 + xd*x0      (Q1 = C^T diag(boo) R)
      yg  = C @ y
      D   = R^T (z*yg) + Q2 y + xd*y         (Q2 = R^T diag(boo) C)
    (boo never touches the device; folded into Q1/Q2 on host)
  - software-pipelined emission: tile t's scatter/gather phases (PE+DVE) are
    interleaved with tile t+1's MLP/z production (PE+ACT) so every engine's
    in-order stream has independent work covering cross-engine latencies.

Data parallel over 8 cores: batch 32768 -> 8 x 4096.
"""

import sys

if "/opt/trn_rl_repo" not in sys.path:
    sys.path.insert(0, "/opt/trn_rl_repo")

import numpy as np

N = 64
H = 256
B = 32768
OFF = 2016
NCORES = 8
B_CORE = B // NCORES   # 4096
F = 512                # batch tile (free dim)
NCHUNK = 16            # 2016 = 16 * 126
CH = OFF // NCHUNK     # 126


def _build_nc(b_core=B_CORE, f=F, reps=1):
    """reps>1 unrolls the whole batch loop `reps` times inside one NEFF
    (same data, same outputs) — used by the timing harness to measure
    steady-state per-pass device time without dispatch overhead."""
    import concourse.bacc as bacc
    import concourse.mybir as mybir
    import concourse.tile as tile

    F32 = mybir.dt.float32
    F32R = mybir.dt.float32r
    Tanh = mybir.ActivationFunctionType.Tanh
    Copy = mybir.ActivationFunctionType.Copy

    ntiles = b_core // f
    assert b_core % f == 0 and f % 128 == 0
    ncol = f // 128

    nc = bacc.Bacc("TRN2", target_bir_lowering=False, debug=False,
                   num_devices=NCORES)

    # --- DRAM tensors ---
    x_d = nc.dram_tensor("x", [b_core, N], F32R, kind="ExternalInput")
    wd1_d = nc.dram_tensor("wd1t", [N, H], F32R, kind="ExternalInput")
    wd2_d = nc.dram_tensor("wd2t", [H, H], F32R, kind="ExternalInput")
    wdo_d = nc.dram_tensor("wdot", [H, N], F32R, kind="ExternalInput")
    wo1_d = nc.dram_tensor("wo1t", [N, H], F32R, kind="ExternalInput")
    wo2_d = nc.dram_tensor("wo2t", [H, H], F32R, kind="ExternalInput")
    woo_d = nc.dram_tensor("woot", [H, OFF], F32R, kind="ExternalInput")
    r_d = nc.dram_tensor("rmat", [OFF, N], F32R, kind="ExternalInput")
    c_d = nc.dram_tensor("cmat", [OFF, N], F32R, kind="ExternalInput")
    # gather lhsT matrices, duplicated on both partition halves for 2x
    # row-group packing (tile_position (0,0) / (64,0))
    rt_d = nc.dram_tensor("rtmat", [128, OFF], F32R, kind="ExternalInput")
    ct_d = nc.dram_tensor("ctmat", [128, OFF], F32R, kind="ExternalInput")
    bl_d = nc.dram_tensor("blmat", [N, N], F32R, kind="ExternalInput")
    blt_d = nc.dram_tensor("bltmat", [N, N], F32R, kind="ExternalInput")
    id_d = nc.dram_tensor("ident", [128, 128], F32R, kind="ExternalInput")
    bd1_d = nc.dram_tensor("bd1", [2, 128, 1], F32, kind="ExternalInput")
    bd2_d = nc.dram_tensor("bd2", [2, 128, 1], F32, kind="ExternalInput")
    bo1_d = nc.dram_tensor("bo1", [2, 128, 1], F32, kind="ExternalInput")
    bo2_d = nc.dram_tensor("bo2", [2, 128, 1], F32, kind="ExternalInput")
    bdo_d = nc.dram_tensor("bdo", [N, 1], F32, kind="ExternalInput")
    out_d = nc.dram_tensor("out", [b_core, N], F32, kind="ExternalOutput")

    with tile.TileContext(nc) as tc:
        with (
            tc.tile_pool(name="wpool", bufs=1) as wp,
            tc.tile_pool(name="apool", bufs=1) as ap,
            tc.tile_pool(name="zpool", bufs=1) as zp,
            tc.tile_pool(name="upool", bufs=1) as up,
            tc.tile_pool(name="iopool", bufs=1) as iop,
            tc.tile_pool(name="psum", bufs=1, space="PSUM") as pp,
        ):
            def wtile(name, src, shape, dt=F32R):
                t = wp.tile(shape, dt, tag=name, name=name, bufs=1)
                nc.sync.dma_start(t[:], src)
                return t

            # light weights first (unblock tile 0's transposes + h1)
            ident = wtile("ident", id_d[:], [128, 128])
            wd1 = wtile("wd1", wd1_d[:], [N, H])
            wo1 = wtile("wo1", wo1_d[:], [N, H])
            bd1 = [wtile(f"bd1_{k}", bd1_d[k], [128, 1], F32) for k in range(2)]
            bo1 = [wtile(f"bo1_{k}", bo1_d[k], [128, 1], F32) for k in range(2)]
            bd2 = [wtile(f"bd2_{k}", bd2_d[k], [128, 1], F32) for k in range(2)]
            bo2 = [wtile(f"bo2_{k}", bo2_d[k], [128, 1], F32) for k in range(2)]
            bdo = wtile("bdo", bdo_d[:], [N, 1], F32)

            # x-input DMA queue (sync ring, ahead of the heavy weights)
            total_tiles = ntiles * reps
            x_tiles = {}

            def emit_xdma(tt):
                if tt >= total_tiles:
                    return
                t = tt % ntiles
                xt = iop.tile([128, ncol, N], F32R, tag="x_in", bufs=3,
                              name=f"x_in{tt}")
                nc.sync.dma_start(
                    xt[:], x_d[t * f:(t + 1) * f, :]
                    .rearrange("(c p) n -> p c n", p=128))
                x_tiles[tt] = xt

            emit_xdma(0)
            emit_xdma(1)

            # heavy weights
            wd2 = [wtile(f"wd2_{k}", wd2_d[k * 128:(k + 1) * 128, :], [128, H])
                   for k in range(2)]
            wo2 = [wtile(f"wo2_{k}", wo2_d[k * 128:(k + 1) * 128, :], [128, H])
                   for k in range(2)]
            wdo = [wtile(f"wdo_{k}", wdo_d[k * 128:(k + 1) * 128, :], [128, N])
                   for k in range(2)]
            woo = [wtile(f"woo_{k}", woo_d[k * 128:(k + 1) * 128, :], [128, OFF])
                   for k in range(2)]
            rtm = wtile("rtm", rt_d[:], [128, OFF])
            ctm = wtile("ctm", ct_d[:], [128, OFF])
            rmat = [wtile(f"rm_{m}", r_d[m * CH:(m + 1) * CH, :], [CH, N])
                    for m in range(NCHUNK)]
            cmat = [wtile(f"cm_{m}", c_d[m * CH:(m + 1) * CH, :], [CH, N])
                    for m in range(NCHUNK)]
            blm = wtile("blm", bl_d[:], [N, N])
            bltm = wtile("bltm", blt_d[:], [N, N])

            # ---- pipeline stages (generators; yield = PE-group boundary) ----
            def stage_ab(t, st):
                """Input, transposes, MLPs, z production for tile t."""
                xt = x_tiles.pop(t)
                emit_xdma(t + 2)
                t = f"r{t}"  # unique name suffix (t may repeat mod ntiles)
                px = pp.tile([N, ncol, 128], F32R, tag="ptr", bufs=2,
                             name=f"px{t}")
                for c in range(ncol):
                    nc.tensor.transpose(px[:, c, :], xt[:, c, :], ident[:])
                yield
                # duplicated on both partition halves (packed gather rhs);
                # engines can't cross partitions -> dup via SBUF->SBUF DMA
                x0T = ap.tile([128, f], F32R, tag="x0T", bufs=2, name=f"x0T{t}")
                pxf = px.rearrange("p c n -> p (c n)")
                nc.scalar.activation(x0T[0:N, :], pxf, Copy)
                nc.sync.dma_start(x0T[N:2 * N, :], x0T[0:N, :])
                st["x0T"] = x0T

                def layer(tag, wts, rhss, biases):
                    outs = []
                    for m in range(2):
                        ph = pp.tile([128, f], F32, tag="ph", bufs=2,
                                     name=f"ph_{tag}{m}_{t}")
                        nk = len(wts)
                        for k in range(nk):
                            nc.tensor.matmul(
                                ph[:], wts[k][:, m * 128:(m + 1) * 128],
                                rhss[k][:, :], start=(k == 0),
                                stop=(k == nk - 1))
                        h = ap.tile([128, f], F32R, tag=f"{tag}{m}", bufs=2,
                                    name=f"{tag}{m}_{t}")
                        nc.scalar.activation(h[:], ph[:], Tanh,
                                             bias=biases[m][:, 0:1])
                        outs.append(h)
                        yield
                    return outs

                h1o = yield from layer("h1o", [wo1], [x0T[0:N]], bo1)
                h1d = yield from layer("h1d", [wd1], [x0T[0:N]], bd1)
                h2o = yield from layer("h2o", wo2, h1o, bo2)
                h2d = yield from layer("h2d", wd2, h1d, bd2)

                # z chunks first (longest pole), diag head after
                z_sb = []
                for m in range(NCHUNK):
                    pz = pp.tile([CH, f], F32, tag="ph", bufs=2,
                                 name=f"pz{m}_{t}")
                    for k in range(2):
                        nc.tensor.matmul(
                            pz[:], woo[k][:, m * CH:(m + 1) * CH], h2o[k][:],
                            start=(k == 0), stop=(k == 1))
                    zt = zp.tile([CH, f], F32R, tag=f"z{m}", bufs=2,
                                 name=f"z{m}_{t}")
                    nc.scalar.activation(zt[:], pz[:], Copy)
                    z_sb.append(zt)
                    yield
                st["z"] = z_sb

                pxd = pp.tile([N, f], F32, tag="ph", bufs=2, name=f"pxd{t}")
                for k in range(2):
                    nc.tensor.matmul(pxd[:], wdo[k][:], h2d[k][:],
                                     start=(k == 0), stop=(k == 1))
                xd = ap.tile([N, f], F32, tag="xd", bufs=2, name=f"xd{t}")
                nc.vector.tensor_scalar_add(xd[:], pxd[:], bdo[:, 0:1])
                st["xd"] = xd
                yield

            def matvec(t, rhs, z_sb, gather_w, scatter_w, q_w, diag_t, name):
                """acc = scatter_w^T (z * gather_w@rhs) + q_w^T rhs[lo]
                + I^T diag_t  (PSUM accumulation; diag term enters as one more
                scatter chunk). Gathers issue in row-packed pairs (partition
                halves 0-63 / 64-127 of rhs and gather_w run concurrently on
                distinct PE row groups); scatters trail their gathers."""
                acc = pp.tile([N, f], F32, tag="acc", bufs=2,
                              name=f"p{name}{t}")
                pending = []

                def emit_scatter():
                    m0, u0 = pending.pop(0)
                    nc.tensor.matmul(acc[:], scatter_w[m0][:], u0[:],
                                     start=False, stop=False,
                                     skip_group_check=True)

                for m in range(0, NCHUNK, 2):
                    pgs = []
                    for i in (0, 1):
                        lo = i * N  # even chunk -> lo half, odd -> hi half
                        pg = pp.tile([CH, f], F32, tag="pg", bufs=2,
                                     name=f"pg_{name}{m + i}_{t}")
                        nc.tensor.matmul(
                            pg[:],
                            gather_w[lo:lo + N, (m + i) * CH:(m + i + 1) * CH],
                            rhs[lo:lo + N, :], start=True, stop=True,
                            tile_position=(lo, 0))
                        pgs.append(pg)
                    for i in (0, 1):
                        u = up.tile([CH, f], F32R, tag="u", bufs=4,
                                    name=f"u_{name}{m + i}_{t}")
                        nc.vector.tensor_mul(u[:], z_sb[m + i][:], pgs[i][:])
                        pending.append((m + i, u))
                    if m == 0:
                        # first member of the accumulation group
                        nc.tensor.matmul(acc[:], q_w[:], rhs[0:N, :],
                                         start=True, stop=False,
                                         skip_group_check=True)
                    while len(pending) > 2:
                        emit_scatter()
                    yield
                while pending:
                    emit_scatter()
                # diag contribution closes the accumulation group
                nc.tensor.matmul(acc[:], ident[0:N, 0:N], diag_t[:],
                                 start=False, stop=True, skip_group_check=True)
                yield
                return acc

            def stage_cd(tt, st):
                """Both L matvecs + output for tile tt."""
                t_out = tt % ntiles
                t = f"r{tt}"  # unique name suffix
                x0T, z_sb, xd = st["x0T"], st["z"], st["xd"]
                t1 = ap.tile([N, f], F32R, tag="t1", bufs=2, name=f"t1_{t}")
                nc.vector.tensor_mul(t1[:], xd[:], x0T[0:N, :])

                py = yield from matvec(t, x0T, z_sb, rtm, cmat, blm, t1, "y")
                y = ap.tile([128, f], F32R, tag="y", bufs=2, name=f"y{t}")
                nc.scalar.activation(y[0:N, :], py[:], Copy)
                nc.sync.dma_start(y[N:2 * N, :], y[0:N, :])
                t2 = ap.tile([N, f], F32R, tag="t2", bufs=2, name=f"t2_{t}")
                nc.vector.tensor_mul(t2[:], xd[:], y[0:N, :])
                yield

                pd = yield from matvec(t, y, z_sb, ctm, rmat, bltm, t2, "d")
                dd = ap.tile([N, f], F32R, tag="dd", bufs=2, name=f"dd{t}")
                nc.scalar.activation(dd[:], pd[:], Copy)
                yield

                po = pp.tile([128, ncol, N], F32R, tag="ptr", bufs=2,
                             name=f"po{t}")
                for c in range(ncol):
                    nc.tensor.transpose(po[:, c, :],
                                        dd[:, c * 128:(c + 1) * 128],
                                        ident[:N, :N])
                o_sb = iop.tile([128, ncol, N], F32, tag="o_sb", bufs=2,
                                name=f"o_sb{t}")
                nc.scalar.activation(o_sb[:], po.rearrange("p c n -> p (c n)"),
                                     Copy)
                nc.gpsimd.dma_start(
                    out_d[t_out * f:(t_out + 1) * f, :]
                    .rearrange("(c p) n -> p c n", p=128), o_sb[:])
                yield

            # ---- driver: round-robin CD(t) with AB(t+1) ----
            def drain(g):
                for _ in g:
                    pass

            states = {0: {}}
            drain(stage_ab(0, states[0]))
            for tt in range(total_tiles):
                gens = [stage_cd(tt, states[tt])]
                if tt + 1 < total_tiles:
                    states[tt + 1] = {}
                    gens.append(stage_ab(tt + 1, states[tt + 1]))
                while gens:
                    for g in list(gens):
                        try:
                            next(g)
                        except StopIteration:
                            gens.remove(g)
                del states[tt]

    nc.compile()
    return nc


def _host_constants(Wd1, bd1, Wd2, bd2, Wdo, bdo, Wo1, bo1, Wo2, bo2, Woo, boo):
    """Shared (per-core replicated) input arrays."""
    f32 = np.float32
    rows, cols = np.tril_indices(N, k=-1)
    R = np.zeros((OFF, N), f32)
    R[np.arange(OFF), rows] = 1.0
    C = np.zeros((OFF, N), f32)
    C[np.arange(OFF), cols] = 1.0
    BL = np.zeros((N, N), f32)
    BL[rows, cols] = np.asarray(boo, f32)

    def ct(a):
        return np.ascontiguousarray(a, dtype=f32)

    return {
        "wd1t": ct(np.asarray(Wd1).T), "wd2t": ct(np.asarray(Wd2).T),
        "wdot": ct(np.asarray(Wdo).T), "wo1t": ct(np.asarray(Wo1).T),
        "wo2t": ct(np.asarray(Wo2).T), "woot": ct(np.asarray(Woo).T),
        "rmat": R, "cmat": C,
        "rtmat": ct(np.vstack([R.T, R.T])), "ctmat": ct(np.vstack([C.T, C.T])),
        "blmat": BL, "bltmat": ct(BL.T),
        "ident": np.eye(128, dtype=f32),
        "bd1": ct(np.asarray(bd1).reshape(2, 128, 1)),
        "bd2": ct(np.asarray(bd2).reshape(2, 128, 1)),
        "bo1": ct(np.asarray(bo1).reshape(2, 128, 1)),
        "bo2": ct(np.asarray(bo2).reshape(2, 128, 1)),
        "bdo": ct(np.asarray(bdo).reshape(N, 1)),
    }


_NC_CACHE = {}


def get_nc(b_core=B_CORE, f=F, reps=1):
    key = (b_core, f, reps)
    if key not in _NC_CACHE:
        _NC_CACHE[key] = _build_nc(b_core, f, reps)
    return _NC_CACHE[key]


def make_in_maps(input, **params):
    shared = _host_constants(**params)
    x = np.ascontiguousarray(np.asarray(input), dtype=np.float32)
    assert x.shape == (B, N)
    return [dict(shared, x=x[c * B_CORE:(c + 1) * B_CORE]) for c in range(NCORES)]


def kernel(input, **params):
    from concourse import bass_utils

    nc = get_nc()
    in_maps = make_in_maps(input, **params)
    res = bass_utils.run_bass_kernel_spmd(nc, in_maps,
                                          core_ids=list(range(NCORES)))
    return np.concatenate([r["out"] for r in res.results], axis=0)

